# revision 16
# baseline (speedup 1.0000x reference)
"""Trainium2 Bass kernel for 16-head MHA (B=2, S=2048, D=1024, E=64).

Sharding: 8 cores = 2 batches x 4 head-groups. Each core computes 4 heads
(2 pairs of 2) for one batch and returns a partial output [2048, 1024]
(sum of its 4 heads' contributions after the output projection) in fp16.
Host sums the 4 partials per batch.

Per-core schedule, paced by the ACT engine's exp cadence (~1.1us per
128x1024 tile, 128 tiles total):
  - PE warm-up burst + ACT table preload ride the initial DMA wait.
  - Input DMA is issued from 4 engine queues, smallest-deadline first, so
    the first S matmul fires as soon as ~3MB have landed.
  - 8 steps of (query-chunk x pair). Step s runs S^T+exp per key block;
    the AV matmuls of step s-1, the norm of step s-1, one quarter of the
    remaining projections, and the output projection of query-chunk
    (s-3)//2 are slotted into its kb loop, budgeted under the exp cadence.
  - softmax denominators via the [V|1] ones column; normalization uses
    reciprocal_approx_fast + gpsimd partition-broadcast, multiplied
    straight out of PSUM into fp16 O^T.
"""

import sys

sys.path.insert(0, "/opt/trn_rl_repo")

import numpy as np

import concourse.bass as bass
import concourse.bacc as bacc
import concourse.mybir as mybir
from concourse import tile
from concourse.bass_interp import get_hw_module
from concourse.bass_utils import run_bass_kernel_spmd

F16 = mybir.dt.float16
F32 = mybir.dt.float32
BF16 = mybir.dt.bfloat16

N_CORES = 8
T = 2048          # tokens per core (one batch)
D = 1024          # model dim
E = 64            # head dim
QC = 512          # query chunk
NQ = T // QC      # 4 query chunks
KB = 128          # key block
NKB = T // KB     # 16 key blocks
ND = D // 128     # 8 contraction chunks for projections

_CACHE = {}


def _build():
    nc = bacc.Bacc("TRN2", target_bir_lowering=False, debug=False,
                   num_devices=N_CORES)

    xqT = nc.dram_tensor("xqT", [D, T], F16, kind="ExternalInput").ap()
    xkT = nc.dram_tensor("xkT", [D, T], F16, kind="ExternalInput").ap()
    xvT = nc.dram_tensor("xvT", [D, T], F16, kind="ExternalInput").ap()
    # per-pair packed weights, layout [128, 8*128]: chunk d at cols d*128
    wq = [nc.dram_tensor(f"wq{p}", [128, D], F16, kind="ExternalInput").ap()
          for p in range(2)]
    wk = [nc.dram_tensor(f"wk{p}", [128, D], F16, kind="ExternalInput").ap()
          for p in range(2)]
    # all-4-head V weights for token-major projection: chunk d at cols d*256
    wv4 = nc.dram_tensor("wv4", [128, ND * 256], F16, kind="ExternalInput").ap()
    wo = [nc.dram_tensor(f"wo{p}", [128, D], F16, kind="ExternalInput").ap()
          for p in range(2)]
    pout = nc.dram_tensor("pout", [T, D], F16, kind="ExternalOutput").ap()

    with tile.TileContext(nc) as tc:
        with (
            tc.tile_pool(name="consts", bufs=1) as consts,
            tc.tile_pool(name="persist", bufs=1) as persist,
            tc.tile_pool(name="xs", bufs=1) as xs,
            tc.tile_pool(name="at", bufs=24) as atp,
            tc.tile_pool(name="o2t", bufs=2) as o2tp,
            tc.tile_pool(name="os", bufs=2) as osp,
            tc.tile_pool(name="small", bufs=1) as smallp,
            tc.tile_pool(name="psS", bufs=2, space="PSUM") as psS,
            tc.tile_pool(name="psO", bufs=1, space="PSUM") as psO,
            tc.tile_pool(name="psX", bufs=2, space="PSUM") as psX,
        ):
            # ---- SBUF tiles ----
            wq_sb = [consts.tile([128, D], F16, tag=f"wq{p}", name=f"wq_sb{p}") for p in range(2)]
            wk_sb = [consts.tile([128, D], F16, tag=f"wk{p}", name=f"wk_sb{p}") for p in range(2)]
            wo_sb = [consts.tile([128, D], F16, tag=f"wo{p}", name=f"wo_sb{p}") for p in range(2)]
            wv4_sb = consts.tile([128, ND * 256], F16, tag="wv4", name="wv4_sb")
            qt = [[persist.tile([128, QC], F16, tag=f"qt{p}_{t}", name=f"qt{p}_{t}")
                   for t in range(NQ)] for p in range(2)]
            kt = [persist.tile([128, T], F16, tag=f"kt{p}", name=f"kt{p}") for p in range(2)]
            # token(key)-major [V | 1] per (head, key-block): [128, 65] each
            v2 = [[persist.tile([128, 65], BF16, tag=f"v2_{h}_{b}", name=f"v2_{h}_{b}")
                   for b in range(NKB)] for h in range(4)]
            ones_f32 = consts.tile([128, 1], F32, tag="ones", name="ones_f32")
            wu = consts.tile([128, 128], F16, tag="wu", name="wu")
            dummy_act = consts.tile([1, 1], BF16, tag="dummy", name="dummy_act")
            xk = [xs.tile([128, T], F16, tag=f"xk{d}", name=f"xk{d}") for d in range(ND)]
            xq = [xs.tile([128, T], F16, tag=f"xq{d}", name=f"xq{d}") for d in range(ND)]
            xv = [xs.tile([128, T], F16, tag=f"xv{d}", name=f"xv{d}") for d in range(ND)]

            # ---- t=0: warm-up + ACT table preload (ride the DMA wait) ----
            nc.vector.memset(wu[:], 0.125)
            nc.vector.memset(ones_f32[:], 1.0)
            nc.scalar.activation(dummy_act[:], wu[0:1, 0:1],
                                 mybir.ActivationFunctionType.Exp)
            wups = psX.tile([128, QC], F32, tag="x", name="warmup_ps")
            for i in range(34):
                nc.tensor.matmul(wups[:, 0:128], wu[:], wu[:],
                                 start=True, stop=True)

            # ---- DMA phase A: minimal set for the first S, striped over
            # 4 issue queues (each dma_start costs ~620ns of issue time) ----
            qengs = [nc.sync, nc.gpsimd, nc.scalar]

            def dmac(eng, dst, src):
                eng.dma_start(dst, src)

            # K inputs strictly first (kproj leads), then Q
            phaseA = [(wk_sb[0][:], wk[0][:]), (wk_sb[1][:], wk[1][:])]
            for d in range(ND):
                phaseA.append((xk[d][:, 0:QC], xkT[d * 128:(d + 1) * 128, 0:QC]))
            phaseA.append((wq_sb[0][:], wq[0][:]))
            phaseA.append((wq_sb[1][:], wq[1][:]))
            for d in range(ND):
                phaseA.append((xq[d][:, 0:QC], xqT[d * 128:(d + 1) * 128, 0:QC]))
            for i, (dst, src) in enumerate(phaseA):
                dmac(qengs[i % 3], dst, src)

            # v2 ones columns (vector; overlaps remaining DMA wait)
            for h in range(4):
                for b in range(NKB):
                    nc.vector.tensor_copy(v2[h][b][:, 64:65], ones_f32[:])

            # ---- DMA phase B: deadline order, on sync+gpsimd only ----
            phaseB = []

            def addB(x, xT_, c0, c1):
                for d in range(ND):
                    phaseB.append((x[d][:, c0:c1], xT_[d * 128:(d + 1) * 128, c0:c1]))

            addB(xk, xkT, QC, 2 * QC)          # kproj t1 chunks
            addB(xk, xkT, 2 * QC, 3 * QC)      # kproj t2
            addB(xk, xkT, 3 * QC, 4 * QC)      # kproj t3
            phaseB.append((wv4_sb[:], wv4[:]))
            addB(xv, xvT, 0, QC)               # vproj kc0-3
            addB(xv, xvT, QC, 2 * QC)          # vproj kc4-7
            addB(xq, xqT, QC, 2 * QC)          # qproj(1,*)
            addB(xv, xvT, 2 * QC, 4 * QC)      # vproj kc8-15
            phaseB.append((wo_sb[0][:], wo[0][:]))
            phaseB.append((wo_sb[1][:], wo[1][:]))
            addB(xq, xqT, 2 * QC, 3 * QC)      # qproj(2,*)
            addB(xq, xqT, 3 * QC, 4 * QC)      # qproj(3,*)
            for i, (dst, src) in enumerate(phaseB):
                dmac(nc.sync if i % 2 == 0 else nc.gpsimd, dst, src)

            # ---- projection helpers ----
            def chain_pair(specs, pfx):
                """Full 2-interleaved accumulation chains (pre-phase only).
                specs: list of (x, w_sb_p, dst_ap, col_slice)."""
                pss = [psX.tile([128, QC], F32, tag="x", name=f"ps_{pfx}_{i}")
                       for i in range(len(specs))]
                for d in range(ND):
                    for i, (x, w_sb_p, dst, sl) in enumerate(specs):
                        nc.tensor.matmul(
                            pss[i][:], w_sb_p[:, d * 128:(d + 1) * 128],
                            x[d][:, sl], start=(d == 0), stop=(d == ND - 1))
                for i, (x, w_sb_p, dst, sl) in enumerate(specs):
                    nc.vector.tensor_copy(dst, pss[i][:])

            def proj_halves(x, w_sb_p, dst, sl, pfx):
                """One chain split into two ~0.85us closures for slotting."""
                st = {}

                def h1():
                    st["ps"] = psX.tile([128, QC], F32, tag="x", name=f"ps_{pfx}")
                    for d in range(4):
                        nc.tensor.matmul(
                            st["ps"][:], w_sb_p[:, d * 128:(d + 1) * 128],
                            x[d][:, sl], start=(d == 0), stop=False)

                def h2():
                    for d in range(4, ND):
                        nc.tensor.matmul(
                            st["ps"][:], w_sb_p[:, d * 128:(d + 1) * 128],
                            x[d][:, sl], start=False, stop=(d == ND - 1))
                    nc.vector.tensor_copy(dst, st["ps"][:])

                return h1, h2

            def kproj_halves(p, t):
                sl = slice(t * QC, (t + 1) * QC)
                return proj_halves(xk, wk_sb[p], kt[p][:, sl], sl, f"k{p}{t}")

            def qproj_halves(qc_, p):
                sl = slice(qc_ * QC, (qc_ + 1) * QC)
                return proj_halves(xq, wq_sb[p], qt[p][qc_][:], sl, f"q{qc_}{p}")

            def emit_vproj(kc, pr):
                """token-major V projection for key block kc, one head pair."""
                psv = psX.tile([128, 128], F32, tag="x", name=f"psv_{kc}_{pr}")
                for d in range(ND):
                    nc.tensor.matmul(
                        psv[:],
                        xv[d][:, kc * 128:(kc + 1) * 128],
                        wv4_sb[:, d * 256 + pr * 128:d * 256 + (pr + 1) * 128],
                        start=(d == 0), stop=(d == ND - 1))
                for h in range(2):
                    nc.vector.tensor_copy(v2[2 * pr + h][kc][:, 0:64],
                                          psv[:, h * 64:(h + 1) * 64])

            # ---- attention pipeline state ----
            at_all = {}     # (s, kb) -> at tile
            po_all = {}     # s -> [po_h0, po_h1]
            o2t_all = {}    # (qc, p) -> o2t tile
            os_live = {}

            def emit_av(s, kb):
                p = s % 2
                if kb == 0:
                    po_all[s] = [psO.tile([65, QC], F32, tag=f"o{h}",
                                          name=f"po_{s}_{h}") for h in range(2)]
                po = po_all[s]
                at_t = at_all[(s, kb)]
                for h in range(2):
                    nc.tensor.matmul(
                        po[h][:], v2[2 * p + h][kb][:],
                        at_t[:, h * QC:(h + 1) * QC],
                        start=(kb == 0), stop=(kb == NKB - 1))

            def emit_norm(s):
                qc_, p = s // 2, s % 2
                po = po_all[s]
                o2t_t = o2tp.tile([128, QC], F16, tag=f"o2t{p}",
                                  name=f"o2t_{qc_}_{p}")
                # interleave the two heads' chains so the gpsimd broadcasts
                # overlap the DVE work
                d_sb = [smallp.tile([1, QC], F32, tag=f"d{h}", name=f"d_{s}_{h}")
                        for h in range(2)]
                r = [smallp.tile([1, QC], F32, tag=f"r{h}", name=f"r_{s}_{h}")
                     for h in range(2)]
                rb = [smallp.tile([64, QC], F32, tag=f"rb{h}", name=f"rb_{s}_{h}")
                      for h in range(2)]
                for h in range(2):
                    nc.vector.tensor_copy(d_sb[h][:], po[h][64:65, :])
                for h in range(2):
                    nc.vector.reciprocal_approx_fast(r[h][:], d_sb[h][:])
                for h in range(2):
                    nc.gpsimd.partition_broadcast(rb[h][:], r[h][:])
                for h in range(2):
                    nc.vector.tensor_mul(
                        o2t_t[h * 64:(h + 1) * 64, :],
                        po[h][0:64, :], rb[h][:])
                o2t_all[(qc_, p)] = o2t_t
                del po_all[s]

            def emit_outproj_group(oqc, sub, oc):
                q0 = oqc * QC
                if oc == 0:
                    os_live[(oqc, sub)] = osp.tile(
                        [128, D], F16, tag="os", name=f"os_{oqc}_{sub}")
                ost = os_live[(oqc, sub)]
                pp = psX.tile([128, 512], F32, tag="x", name=f"pp_{oqc}_{sub}_{oc}")
                for p in range(2):
                    nc.tensor.matmul(
                        pp[:],
                        o2t_all[(oqc, p)][:, sub * 128:(sub + 1) * 128],
                        wo_sb[p][:, oc * 512:(oc + 1) * 512],
                        start=(p == 0), stop=(p == 1))
                nc.vector.tensor_copy(ost[:, oc * 512:(oc + 1) * 512], pp[:])
                if oc == 1:
                    nc.sync.dma_start(
                        pout[q0 + sub * 128:q0 + (sub + 1) * 128, :],
                        ost[:])
                    del os_live[(oqc, sub)]

            # ---- pre-phase: K chunk0 + Q chunk0 for both pairs ----
            chain_pair([(xk, wk_sb[0], kt[0][:, 0:QC], slice(0, QC)),
                        (xk, wk_sb[1], kt[1][:, 0:QC], slice(0, QC))], "kpre")
            chain_pair([(xq, wq_sb[0], qt[0][0][:], slice(0, QC)),
                        (xq, wq_sb[1], qt[1][0][:], slice(0, QC))], "qpre")

            # ---- insert schedule ----
            from collections import defaultdict
            inserts = defaultdict(list)

            def put(s, kb, fn):
                inserts[(s, kb)].append(fn)

            # step 0: remaining K chunks + first V blocks.  Split-chain
            # halves: at most one psX allocation lands between h1 and h2.
            for i, (p, t) in enumerate([(0, 1), (0, 2), (0, 3)]):
                h1, h2 = kproj_halves(p, t)
                put(0, (2, 5, 7)[i], h1)
                put(0, (3, 6, 8)[i], h2)
            put(0, 9, lambda: emit_vproj(0, 0))
            put(0, 10, lambda: emit_vproj(1, 0))
            h1, h2 = kproj_halves(1, 1)
            put(0, 11, h1)
            put(0, 12, lambda: emit_vproj(2, 0))
            put(0, 13, h2)
            put(0, 13, lambda: emit_vproj(0, 1))
            put(0, 14, lambda: emit_vproj(3, 0))
            put(0, 14, lambda: emit_vproj(1, 1))
            put(0, 15, lambda: emit_vproj(2, 1))
            put(0, 15, lambda: emit_vproj(3, 1))
            # step 1: rest of kproj pair1 (halves, no interleaved psX),
            # pair-0 V blocks, qproj for step 2
            h1, h2 = kproj_halves(1, 2)
            put(1, 0, h1)
            put(1, 1, h2)
            h1, h2 = kproj_halves(1, 3)
            put(1, 2, h1)
            put(1, 3, h2)
            for kc in range(4, 16):
                put(1, 4 + (kc - 4) // 2, lambda kc=kc: emit_vproj(kc, 0))
            h1, h2 = qproj_halves(1, 0)
            put(1, 10, h1)
            put(1, 12, h2)
            # step 2: pair-1 V blocks + qproj
            for kc in range(4, 16):
                put(2, 2 + (kc - 4) // 2, lambda kc=kc: emit_vproj(kc, 1))
            h1, h2 = qproj_halves(1, 1)
            put(2, 9, h1)
            put(2, 11, h2)
            # steps 3..6: one qproj per step for step s+1
            qsched = {3: (2, 0), 4: (2, 1), 5: (3, 0), 6: (3, 1)}
            for s, (qc_, p) in qsched.items():
                h1, h2 = qproj_halves(qc_, p)
                put(s, 10, h1)
                put(s, 12, h2)
            # AV + norm: AV(0) spills into step 2, AV(1) sits late in
            # step 2, AV(s-1) for s>=3 runs dense at slots 0..10 with the
            # tail + norm at slot 13
            for j in range(10):
                put(1, j + 3, lambda j=j: emit_av(0, j))

            def av0_tail():
                for j in range(10, NKB):
                    emit_av(0, j)
                emit_norm(0)
            put(2, 1, av0_tail)
            for j in range(8):
                put(2, j + 5, lambda j=j: emit_av(1, j))

            def av1_tail():
                for j in range(8, NKB):
                    emit_av(1, j)
                emit_norm(1)
            put(2, 12, av1_tail)
            for s in range(3, 8):
                for j in range(11):
                    put(s, j, lambda s=s, j=j: emit_av(s - 1, j))

                def av_tail(s=s):
                    for j in range(11, NKB):
                        emit_av(s - 1, j)
                    emit_norm(s - 1)
                put(s, 11, av_tail)
            # outproj(oqc): 6 groups in step 2*oqc+3, 2 in step 2*oqc+4
            # (oqc<=1); outproj(2) fully in step 7; outproj(3) in the tail
            groups = [(i, j) for i in range(4) for j in range(2)]
            for oqc in range(3):
                for g, (sub, oc) in enumerate(groups):
                    if oqc == 2:
                        s, slot = 7, (1, 3, 5, 7, 9, 13, 14, 15)[g]
                    elif g < 6:
                        s, slot = 2 * oqc + 3, (1, 3, 5, 7, 9, 13)[g]
                    else:
                        s, slot = 2 * oqc + 4, (1, 3)[g - 6]
                    put(s, slot, lambda oqc=oqc, sub=sub, oc=oc:
                        emit_outproj_group(oqc, sub, oc))

            # ---- 8-step S/exp pipeline with slotted inserts ----
            steps = [(qc_, p) for qc_ in range(NQ) for p in range(2)]
            for s, (qc_, p) in enumerate(steps):
                for kb in range(NKB):
                    k0 = kb * KB
                    ps = psS.tile([128, 2 * QC], F32, tag="s", name=f"s_{s}_{kb}")
                    nc.tensor.matmul(
                        ps[:, 0:QC],
                        kt[p][0:64, k0:k0 + KB],
                        qt[p][qc_][0:64, :],
                        start=True, stop=True, tile_position=(0, 0))
                    nc.tensor.matmul(
                        ps[:, QC:2 * QC],
                        kt[p][64:128, k0:k0 + KB],
                        qt[p][qc_][64:128, :],
                        start=True, stop=True, tile_position=(64, 0))
                    at_t = atp.tile([128, 2 * QC], BF16, tag="at",
                                    name=f"at_{s}_{kb}")
                    nc.scalar.activation(
                        at_t[:], ps[:], mybir.ActivationFunctionType.Exp)
                    at_all[(s, kb)] = at_t
                    for fn in inserts.get((s, kb), ()):
                        fn()

            # ---- tail: AV + norm of step 7, then outproj(3) ----
            for j in range(NKB):
                emit_av(7, j)
            emit_norm(7)
            for sub in range(4):
                for oc in range(2):
                    emit_outproj_group(3, sub, oc)

    nc.compile()
    nc.m = get_hw_module(nc.m)
    return nc


def _pack_w(w_pair):
    # w_pair: [2, 1024, 64] -> [1024, 128] -> chunk-major [128, 8*128]
    w = np.concatenate([w_pair[0], w_pair[1]], axis=1)          # [1024, 128]
    return np.ascontiguousarray(
        w.reshape(ND, 128, 128).transpose(1, 0, 2).reshape(128, D))


def _pack_wv4(w_quad):
    # w_quad: [4, 1024, 64] -> [1024, 256] -> chunk-major [128, 8*256]
    w = np.concatenate([w_quad[h] for h in range(4)], axis=1)   # [1024, 256]
    return np.ascontiguousarray(
        w.reshape(ND, 128, 256).transpose(1, 0, 2).reshape(128, ND * 256))


def _pack_wo(wo_pair):
    # wo_pair: [2, 64, 1024] -> [128, 1024]
    return np.ascontiguousarray(np.concatenate([wo_pair[0], wo_pair[1]], axis=0))


def kernel(q, k, v, W_query, W_key, W_val, W_out, _trace=False):
    q = np.asarray(q, dtype=np.float32)
    k = np.asarray(k, dtype=np.float32)
    v = np.asarray(v, dtype=np.float32)
    W_query = np.asarray(W_query, dtype=np.float32)
    W_key = np.asarray(W_key, dtype=np.float32)
    W_val = np.asarray(W_val, dtype=np.float32)
    W_out = np.asarray(W_out, dtype=np.float32)

    if "nc" not in _CACHE:
        _CACHE["nc"] = _build()
    nc = _CACHE["nc"]

    norm = 1.0 / np.sqrt(E)
    xT = {}
    for b in range(2):
        xT[("q", b)] = np.ascontiguousarray(q[b].T).astype(np.float16)
        xT[("k", b)] = np.ascontiguousarray(k[b].T).astype(np.float16)
        xT[("v", b)] = np.ascontiguousarray(v[b].T).astype(np.float16)

    in_maps = []
    for c in range(N_CORES):
        b, g = c // 4, c % 4
        hs = [4 * g, 4 * g + 1, 4 * g + 2, 4 * g + 3]
        m = {
            "xqT": xT[("q", b)], "xkT": xT[("k", b)], "xvT": xT[("v", b)],
            "wv4": _pack_wv4(W_val[hs]).astype(np.float16),
        }
        for p in range(2):
            hp = hs[2 * p:2 * p + 2]
            m[f"wq{p}"] = _pack_w(W_query[hp] * norm).astype(np.float16)
            m[f"wk{p}"] = _pack_w(W_key[hp]).astype(np.float16)
            m[f"wo{p}"] = _pack_wo(W_out[hp]).astype(np.float16)
        in_maps.append(m)

    res = run_bass_kernel_spmd(nc, in_maps, list(range(N_CORES)),
                               trace=_trace)
    parts = [res.results[c]["pout"].astype(np.float32) for c in range(N_CORES)]
    out = np.stack([
        parts[0] + parts[1] + parts[2] + parts[3],
        parts[4] + parts[5] + parts[6] + parts[7],
    ])
    if _trace:
        _CACHE["last_result"] = res
    return out


# revision 19
# speedup vs baseline: 1.0074x; 1.0074x over previous
"""Trainium2 Bass kernel for 16-head MHA (B=2, S=2048, D=1024, E=64).

Sharding: 8 cores = 2 batches x 4 head-groups. Each core computes 4 heads
(2 pairs of 2) for one batch and returns a partial output [2048, 1024]
(sum of its 4 heads' contributions after the output projection) in fp16.
Host sums the 4 partials per batch.

Per-core schedule, paced by the ACT engine's exp cadence (~1.1us per
128x1024 tile, 128 tiles total):
  - PE warm-up burst + ACT table preload ride the initial DMA wait.
  - Input DMA is issued from 4 engine queues, smallest-deadline first, so
    the first S matmul fires as soon as ~3MB have landed.
  - 8 steps of (query-chunk x pair). Step s runs S^T+exp per key block;
    the AV matmuls of step s-1, the norm of step s-1, one quarter of the
    remaining projections, and the output projection of query-chunk
    (s-3)//2 are slotted into its kb loop, budgeted under the exp cadence.
  - softmax denominators via the [V|1] ones column; normalization uses
    reciprocal_approx_fast + gpsimd partition-broadcast, multiplied
    straight out of PSUM into fp16 O^T.
"""

import sys

sys.path.insert(0, "/opt/trn_rl_repo")

import numpy as np

import concourse.bass as bass
import concourse.bacc as bacc
import concourse.mybir as mybir
from concourse import tile
from concourse.tile_rust import add_dep_helper
from concourse.bass_interp import get_hw_module
from concourse.bass_utils import run_bass_kernel_spmd

F16 = mybir.dt.float16
F32 = mybir.dt.float32
BF16 = mybir.dt.bfloat16

N_CORES = 8
T = 2048          # tokens per core (one batch)
D = 1024          # model dim
E = 64            # head dim
QC = 512          # query chunk
NQ = T // QC      # 4 query chunks
KB = 128          # key block
NKB = T // KB     # 16 key blocks
ND = D // 128     # 8 contraction chunks for projections

_CACHE = {}


def _build():
    nc = bacc.Bacc("TRN2", target_bir_lowering=False, debug=False,
                   num_devices=N_CORES)

    xqT = nc.dram_tensor("xqT", [D, T], F16, kind="ExternalInput").ap()
    xkT = nc.dram_tensor("xkT", [D, T], F16, kind="ExternalInput").ap()
    xvT = nc.dram_tensor("xvT", [D, T], F16, kind="ExternalInput").ap()
    # per-pair packed weights, layout [128, 8*128]: chunk d at cols d*128
    wq = [nc.dram_tensor(f"wq{p}", [128, D], F16, kind="ExternalInput").ap()
          for p in range(2)]
    wk = [nc.dram_tensor(f"wk{p}", [128, D], F16, kind="ExternalInput").ap()
          for p in range(2)]
    # all-4-head V weights for token-major projection: chunk d at cols d*256
    wv4 = nc.dram_tensor("wv4", [128, ND * 256], F16, kind="ExternalInput").ap()
    wo = [nc.dram_tensor(f"wo{p}", [128, D], F16, kind="ExternalInput").ap()
          for p in range(2)]
    pout = nc.dram_tensor("pout", [T, D], F16, kind="ExternalOutput").ap()

    with tile.TileContext(nc) as tc:
        with (
            tc.tile_pool(name="consts", bufs=1) as consts,
            tc.tile_pool(name="persist", bufs=1) as persist,
            tc.tile_pool(name="xs", bufs=1) as xs,
            tc.tile_pool(name="at", bufs=24) as atp,
            tc.tile_pool(name="o2t", bufs=2) as o2tp,
            tc.tile_pool(name="os", bufs=2) as osp,
            tc.tile_pool(name="small", bufs=1) as smallp,
            tc.tile_pool(name="psS", bufs=2, space="PSUM") as psS,
            tc.tile_pool(name="psO", bufs=1, space="PSUM") as psO,
            tc.tile_pool(name="psX", bufs=2, space="PSUM") as psX,
        ):
            # ---- SBUF tiles ----
            wq_sb = [consts.tile([128, D], F16, tag=f"wq{p}", name=f"wq_sb{p}") for p in range(2)]
            wk_sb = [consts.tile([128, D], F16, tag=f"wk{p}", name=f"wk_sb{p}") for p in range(2)]
            wo_sb = [consts.tile([128, D], F16, tag=f"wo{p}", name=f"wo_sb{p}") for p in range(2)]
            wv4_sb = consts.tile([128, ND * 256], F16, tag="wv4", name="wv4_sb")
            qt = [[persist.tile([128, QC], F16, tag=f"qt{p}_{t}", name=f"qt{p}_{t}")
                   for t in range(NQ)] for p in range(2)]
            kt = [persist.tile([128, T], F16, tag=f"kt{p}", name=f"kt{p}") for p in range(2)]
            # token(key)-major [V | 1] per (head, key-block): [128, 65] each
            v2 = [[persist.tile([128, 65], BF16, tag=f"v2_{h}_{b}", name=f"v2_{h}_{b}")
                   for b in range(NKB)] for h in range(4)]
            ones_f32 = consts.tile([128, 1], F32, tag="ones", name="ones_f32")
            wu = consts.tile([128, 128], F16, tag="wu", name="wu")
            dummy_act = consts.tile([1, 1], BF16, tag="dummy", name="dummy_act")
            xk = [xs.tile([128, T], F16, tag=f"xk{d}", name=f"xk{d}") for d in range(ND)]
            xq = [xs.tile([128, T], F16, tag=f"xq{d}", name=f"xq{d}") for d in range(ND)]
            xv = [xs.tile([128, T], F16, tag=f"xv{d}", name=f"xv{d}") for d in range(ND)]

            # ---- t=0: warm-up + ACT table preload (ride the DMA wait) ----
            nc.vector.memset(wu[:], 0.125)
            nc.vector.memset(ones_f32[:], 1.0)
            nc.scalar.activation(dummy_act[:], wu[0:1, 0:1],
                                 mybir.ActivationFunctionType.Exp)
            wups = psX.tile([128, QC], F32, tag="x", name="warmup_ps")
            for i in range(44):
                nc.tensor.matmul(wups[:, 0:128], wu[:], wu[:],
                                 start=True, stop=True)

            # ---- DMA phase A: minimal set for the first S, striped over
            # 4 issue queues (each dma_start costs ~620ns of issue time) ----
            qengs = [nc.sync, nc.gpsimd, nc.scalar]

            def dmac(eng, dst, src):
                return eng.dma_start(dst, src)

            # K inputs strictly first (kproj leads), then Q
            phaseA = [(wk_sb[0][:], wk[0][:]), (wk_sb[1][:], wk[1][:])]
            for d in range(ND):
                phaseA.append((xk[d][:, 0:QC], xkT[d * 128:(d + 1) * 128, 0:QC]))
            phaseA.append((wq_sb[0][:], wq[0][:]))
            phaseA.append((wq_sb[1][:], wq[1][:]))
            for d in range(ND):
                phaseA.append((xq[d][:, 0:QC], xqT[d * 128:(d + 1) * 128, 0:QC]))
            lastA = {}
            for i, (dst, src) in enumerate(phaseA):
                lastA[i % 3] = dmac(qengs[i % 3], dst, src)

            # v2 ones columns (vector; overlaps remaining DMA wait)
            for h in range(4):
                for b in range(NKB):
                    nc.vector.tensor_copy(v2[h][b][:, 64:65], ones_f32[:])

            # ---- DMA phase B: deadline order, on sync+gpsimd only ----
            phaseB = []

            def addB(x, xT_, c0, c1):
                for d in range(ND):
                    phaseB.append((x[d][:, c0:c1], xT_[d * 128:(d + 1) * 128, c0:c1]))

            addB(xk, xkT, QC, 2 * QC)          # kproj t1 chunks
            addB(xk, xkT, 2 * QC, 3 * QC)      # kproj t2
            addB(xk, xkT, 3 * QC, 4 * QC)      # kproj t3
            phaseB.append((wv4_sb[:], wv4[:]))
            addB(xv, xvT, 0, QC)               # vproj kc0-3
            addB(xv, xvT, QC, 2 * QC)          # vproj kc4-7
            addB(xq, xqT, QC, 2 * QC)          # qproj(1,*)
            addB(xv, xvT, 2 * QC, 4 * QC)      # vproj kc8-15
            phaseB.append((wo_sb[0][:], wo[0][:]))
            phaseB.append((wo_sb[1][:], wo[1][:]))
            addB(xq, xqT, 2 * QC, 3 * QC)      # qproj(2,*)
            addB(xq, xqT, 3 * QC, 4 * QC)      # qproj(3,*)
            gated = set()
            for i, (dst, src) in enumerate(phaseB):
                eng = nc.sync if i % 2 == 0 else nc.gpsimd
                ins = dmac(eng, dst, src)
                if i < 2:
                    # phase B competes with phase A for HBM bandwidth;
                    # hold it back until the critical set has landed
                    for a in lastA.values():
                        add_dep_helper(ins.ins, a.ins, sync=True,
                                       reason="phaseB after phaseA")

            # ---- projection helpers ----
            def chain_pair(specs, pfx):
                """Full 2-interleaved accumulation chains (pre-phase only).
                specs: list of (x, w_sb_p, dst_ap, col_slice)."""
                pss = [psX.tile([128, QC], F32, tag="x", name=f"ps_{pfx}_{i}")
                       for i in range(len(specs))]
                for d in range(ND):
                    for i, (x, w_sb_p, dst, sl) in enumerate(specs):
                        nc.tensor.matmul(
                            pss[i][:], w_sb_p[:, d * 128:(d + 1) * 128],
                            x[d][:, sl], start=(d == 0), stop=(d == ND - 1))
                for i, (x, w_sb_p, dst, sl) in enumerate(specs):
                    nc.vector.tensor_copy(dst, pss[i][:])

            def proj_chain(x, w_sb_p, dst, sl, pfx):
                """Self-contained 8-matmul accumulation chain (one closure:
                alloc + matmuls + copy, so psX stays strictly sequential)."""
                def fn():
                    ps = psX.tile([128, QC], F32, tag="x", name=f"ps_{pfx}")
                    for d in range(ND):
                        nc.tensor.matmul(
                            ps[:], w_sb_p[:, d * 128:(d + 1) * 128],
                            x[d][:, sl], start=(d == 0), stop=(d == ND - 1))
                    nc.vector.tensor_copy(dst, ps[:])
                return fn

            def kproj1(p, t):
                sl = slice(t * QC, (t + 1) * QC)
                return proj_chain(xk, wk_sb[p], kt[p][:, sl], sl, f"k{p}{t}")

            def qproj1(qc_, p):
                sl = slice(qc_ * QC, (qc_ + 1) * QC)
                return proj_chain(xq, wq_sb[p], qt[p][qc_][:], sl, f"q{qc_}{p}")

            def emit_vproj(kc, pr):
                """token-major V projection for key block kc, one head pair."""
                psv = psX.tile([128, 128], F32, tag="x", name=f"psv_{kc}_{pr}")
                for d in range(ND):
                    nc.tensor.matmul(
                        psv[:],
                        xv[d][:, kc * 128:(kc + 1) * 128],
                        wv4_sb[:, d * 256 + pr * 128:d * 256 + (pr + 1) * 128],
                        start=(d == 0), stop=(d == ND - 1))
                for h in range(2):
                    nc.vector.tensor_copy(v2[2 * pr + h][kc][:, 0:64],
                                          psv[:, h * 64:(h + 1) * 64])

            # ---- attention pipeline state ----
            at_all = {}     # (s, kb) -> at tile
            po_all = {}     # s -> [po_h0, po_h1]
            o2t_all = {}    # (qc, p) -> o2t tile
            os_live = {}

            def emit_av(s, kb):
                p = s % 2
                if kb == 0:
                    po_all[s] = [psO.tile([65, QC], F32, tag=f"o{h}",
                                          name=f"po_{s}_{h}") for h in range(2)]
                po = po_all[s]
                at_t = at_all[(s, kb)]
                for h in range(2):
                    nc.tensor.matmul(
                        po[h][:], v2[2 * p + h][kb][:],
                        at_t[:, h * QC:(h + 1) * QC],
                        start=(kb == 0), stop=(kb == NKB - 1))

            def emit_norm(s):
                qc_, p = s // 2, s % 2
                po = po_all[s]
                o2t_t = o2tp.tile([128, QC], F16, tag=f"o2t{p}",
                                  name=f"o2t_{qc_}_{p}")
                # interleave the two heads' chains so the gpsimd broadcasts
                # overlap the DVE work
                d_sb = [smallp.tile([1, QC], F32, tag=f"d{h}", name=f"d_{s}_{h}")
                        for h in range(2)]
                r = [smallp.tile([1, QC], F32, tag=f"r{h}", name=f"r_{s}_{h}")
                     for h in range(2)]
                rb = [smallp.tile([64, QC], F32, tag=f"rb{h}", name=f"rb_{s}_{h}")
                      for h in range(2)]
                for h in range(2):
                    nc.vector.tensor_copy(d_sb[h][:], po[h][64:65, :])
                for h in range(2):
                    nc.vector.reciprocal_approx_fast(r[h][:], d_sb[h][:])
                for h in range(2):
                    nc.gpsimd.partition_broadcast(rb[h][:], r[h][:])
                for h in range(2):
                    nc.vector.tensor_mul(
                        o2t_t[h * 64:(h + 1) * 64, :],
                        po[h][0:64, :], rb[h][:])
                o2t_all[(qc_, p)] = o2t_t
                del po_all[s]

            def emit_outproj_group(oqc, sub, oc):
                q0 = oqc * QC
                if oc == 0:
                    os_live[(oqc, sub)] = osp.tile(
                        [128, D], F16, tag="os", name=f"os_{oqc}_{sub}")
                ost = os_live[(oqc, sub)]
                pp = psX.tile([128, 512], F32, tag="x", name=f"pp_{oqc}_{sub}_{oc}")
                for p in range(2):
                    nc.tensor.matmul(
                        pp[:],
                        o2t_all[(oqc, p)][:, sub * 128:(sub + 1) * 128],
                        wo_sb[p][:, oc * 512:(oc + 1) * 512],
                        start=(p == 0), stop=(p == 1))
                nc.vector.tensor_copy(ost[:, oc * 512:(oc + 1) * 512], pp[:])
                if oc == 1:
                    nc.sync.dma_start(
                        pout[q0 + sub * 128:q0 + (sub + 1) * 128, :],
                        ost[:])
                    del os_live[(oqc, sub)]

            # ---- pre-phase: K chunk0 + Q chunk0 for both pairs ----
            chain_pair([(xk, wk_sb[0], kt[0][:, 0:QC], slice(0, QC)),
                        (xk, wk_sb[1], kt[1][:, 0:QC], slice(0, QC))], "kpre")
            chain_pair([(xq, wq_sb[0], qt[0][0][:], slice(0, QC)),
                        (xq, wq_sb[1], qt[1][0][:], slice(0, QC))], "qpre")

            # ---- insert schedule ----
            from collections import defaultdict
            inserts = defaultdict(list)

            def put(s, kb, fn):
                inserts[(s, kb)].append(fn)

            # step 0: remaining K chunks (atomic chains, DMA-paced)
            # + first V blocks
            for i, (p, t) in enumerate([(0, 1), (0, 2), (0, 3),
                                        (1, 1), (1, 2), (1, 3)]):
                put(0, 2 * i + 2, kproj1(p, t))
            put(0, 13, lambda: emit_vproj(0, 0))
            put(0, 14, lambda: emit_vproj(1, 0))
            put(0, 14, lambda: emit_vproj(2, 0))
            put(0, 15, lambda: emit_vproj(3, 0))
            put(0, 15, lambda: emit_vproj(0, 1))
            # step 1: rest of V pair-0 + AV(0) + qproj for step 2
            put(1, 0, lambda: emit_vproj(1, 1))
            put(1, 0, lambda: emit_vproj(2, 1))
            put(1, 1, lambda: emit_vproj(3, 1))
            for kc in range(4, 16):
                put(1, 1 + (kc - 4) // 2, lambda kc=kc: emit_vproj(kc, 0))
            for j in range(10):
                put(1, j + 3, lambda j=j: emit_av(0, j))
            put(1, 13, qproj1(1, 0))
            # step 2: AV(0) tail + norm(0), V pair-1, AV(1), qproj
            def av0_tail(lo, hi, with_norm):
                def fn():
                    for j in range(lo, hi):
                        emit_av(0, j)
                    if with_norm:
                        emit_norm(0)
                return fn
            put(2, 0, av0_tail(10, 12, False))
            put(2, 1, av0_tail(12, 14, False))
            put(2, 2, av0_tail(14, 16, True))
            for kc in range(4, 16):
                put(2, 3 + (kc - 4) // 2, lambda kc=kc: emit_vproj(kc, 1))
            for j in range(16):
                put(2, 6 + j // 2, lambda j=j: emit_av(1, j))
            put(2, 13, lambda: emit_norm(1))
            put(2, 14, qproj1(1, 1))
            # steps 3..7: AV(s-1) dense up front (2/slot), norm at slot 8,
            # outproj + qproj in the back half
            for s in range(3, 8):
                for j in range(16):
                    put(s, j // 2, lambda s=s, j=j: emit_av(s - 1, j))
                put(s, 8, lambda s=s: emit_norm(s - 1))
            for s, (qc_, p) in {3: (2, 0), 4: (2, 1),
                                5: (3, 0), 6: (3, 1)}.items():
                put(s, 11, qproj1(qc_, p))
            # outproj(oqc): 6 groups in step 2*oqc+3, 2 in step 2*oqc+4
            # (oqc<=1); outproj(2) fully in step 7; outproj(3) in the tail
            groups = [(i, j) for i in range(4) for j in range(2)]
            for oqc in range(3):
                for g, (sub, oc) in enumerate(groups):
                    if oqc == 2:
                        s, slot = 7, (9, 10, 11, 12, 13, 14, 15, 15)[g]
                    elif g < 6:
                        s, slot = 2 * oqc + 3, (9, 10, 12, 13, 14, 15)[g]
                    else:
                        s, slot = 2 * oqc + 4, (9, 12)[g - 6]
                    put(s, slot, lambda oqc=oqc, sub=sub, oc=oc:
                        emit_outproj_group(oqc, sub, oc))

            # ---- 8-step S/exp pipeline with slotted inserts ----
            steps = [(qc_, p) for qc_ in range(NQ) for p in range(2)]
            for s, (qc_, p) in enumerate(steps):
                for kb in range(NKB):
                    k0 = kb * KB
                    ps = psS.tile([128, 2 * QC], F32, tag="s", name=f"s_{s}_{kb}")
                    nc.tensor.matmul(
                        ps[:, 0:QC],
                        kt[p][0:64, k0:k0 + KB],
                        qt[p][qc_][0:64, :],
                        start=True, stop=True, tile_position=(0, 0))
                    nc.tensor.matmul(
                        ps[:, QC:2 * QC],
                        kt[p][64:128, k0:k0 + KB],
                        qt[p][qc_][64:128, :],
                        start=True, stop=True, tile_position=(64, 0))
                    at_t = atp.tile([128, 2 * QC], BF16, tag="at",
                                    name=f"at_{s}_{kb}")
                    nc.scalar.activation(
                        at_t[:], ps[:], mybir.ActivationFunctionType.Exp)
                    at_all[(s, kb)] = at_t
                    for fn in inserts.get((s, kb), ()):
                        fn()

            # ---- tail: AV + norm of step 7, then outproj(3) ----
            for j in range(NKB):
                emit_av(7, j)
            emit_norm(7)
            for sub in range(4):
                for oc in range(2):
                    emit_outproj_group(3, sub, oc)

    nc.compile()
    nc.m = get_hw_module(nc.m)
    return nc


def _pack_w(w_pair):
    # w_pair: [2, 1024, 64] -> [1024, 128] -> chunk-major [128, 8*128]
    w = np.concatenate([w_pair[0], w_pair[1]], axis=1)          # [1024, 128]
    return np.ascontiguousarray(
        w.reshape(ND, 128, 128).transpose(1, 0, 2).reshape(128, D))


def _pack_wv4(w_quad):
    # w_quad: [4, 1024, 64] -> [1024, 256] -> chunk-major [128, 8*256]
    w = np.concatenate([w_quad[h] for h in range(4)], axis=1)   # [1024, 256]
    return np.ascontiguousarray(
        w.reshape(ND, 128, 256).transpose(1, 0, 2).reshape(128, ND * 256))


def _pack_wo(wo_pair):
    # wo_pair: [2, 64, 1024] -> [128, 1024]
    return np.ascontiguousarray(np.concatenate([wo_pair[0], wo_pair[1]], axis=0))


def kernel(q, k, v, W_query, W_key, W_val, W_out, _trace=False):
    q = np.asarray(q, dtype=np.float32)
    k = np.asarray(k, dtype=np.float32)
    v = np.asarray(v, dtype=np.float32)
    W_query = np.asarray(W_query, dtype=np.float32)
    W_key = np.asarray(W_key, dtype=np.float32)
    W_val = np.asarray(W_val, dtype=np.float32)
    W_out = np.asarray(W_out, dtype=np.float32)

    if "nc" not in _CACHE:
        _CACHE["nc"] = _build()
    nc = _CACHE["nc"]

    norm = 1.0 / np.sqrt(E)
    xT = {}
    for b in range(2):
        xT[("q", b)] = np.ascontiguousarray(q[b].T).astype(np.float16)
        xT[("k", b)] = np.ascontiguousarray(k[b].T).astype(np.float16)
        xT[("v", b)] = np.ascontiguousarray(v[b].T).astype(np.float16)

    in_maps = []
    for c in range(N_CORES):
        b, g = c // 4, c % 4
        hs = [4 * g, 4 * g + 1, 4 * g + 2, 4 * g + 3]
        m = {
            "xqT": xT[("q", b)], "xkT": xT[("k", b)], "xvT": xT[("v", b)],
            "wv4": _pack_wv4(W_val[hs]).astype(np.float16),
        }
        for p in range(2):
            hp = hs[2 * p:2 * p + 2]
            m[f"wq{p}"] = _pack_w(W_query[hp] * norm).astype(np.float16)
            m[f"wk{p}"] = _pack_w(W_key[hp]).astype(np.float16)
            m[f"wo{p}"] = _pack_wo(W_out[hp]).astype(np.float16)
        in_maps.append(m)

    res = run_bass_kernel_spmd(nc, in_maps, list(range(N_CORES)),
                               trace=_trace)
    parts = [res.results[c]["pout"].astype(np.float32) for c in range(N_CORES)]
    out = np.stack([
        parts[0] + parts[1] + parts[2] + parts[3],
        parts[4] + parts[5] + parts[6] + parts[7],
    ])
    if _trace:
        _CACHE["last_result"] = res
    return out


# revision 22
# speedup vs baseline: 1.0369x; 1.0293x over previous
"""Trainium2 Bass kernel for 16-head MHA (B=2, S=2048, D=1024, E=64).

Sharding: 8 cores = 2 batches x 4 head-groups. Each core computes 4 heads
(2 pairs of 2) for one batch and returns a partial output [2048, 1024]
(sum of its 4 heads' contributions after the output projection) in fp16.
Host sums the 4 partials per batch.

Per-core schedule, paced by the ACT engine's exp cadence (~1.1us per
128x1024 tile, 128 tiles total):
  - PE warm-up burst + ACT table preload ride the initial DMA wait.
  - Input DMA is issued from 4 engine queues, smallest-deadline first, so
    the first S matmul fires as soon as ~3MB have landed.
  - 8 steps of (query-chunk x pair). Step s runs S^T+exp per key block;
    the AV matmuls of step s-1, the norm of step s-1, one quarter of the
    remaining projections, and the output projection of query-chunk
    (s-3)//2 are slotted into its kb loop, budgeted under the exp cadence.
  - softmax denominators via the [V|1] ones column; normalization uses
    reciprocal_approx_fast + gpsimd partition-broadcast, multiplied
    straight out of PSUM into fp16 O^T.
"""

import sys

sys.path.insert(0, "/opt/trn_rl_repo")

import numpy as np

import concourse.bass as bass
import concourse.bacc as bacc
import concourse.mybir as mybir
from concourse import tile
from concourse.tile_rust import add_dep_helper
from concourse.bass_interp import get_hw_module
from concourse.bass_utils import run_bass_kernel_spmd

F16 = mybir.dt.float16
F32 = mybir.dt.float32
BF16 = mybir.dt.bfloat16

N_CORES = 8
T = 2048          # tokens per core (one batch)
D = 1024          # model dim
E = 64            # head dim
QC = 512          # query chunk
NQ = T // QC      # 4 query chunks
KB = 128          # key block
NKB = T // KB     # 16 key blocks
ND = D // 128     # 8 contraction chunks for projections

_CACHE = {}


def _build():
    nc = bacc.Bacc("TRN2", target_bir_lowering=False, debug=False,
                   num_devices=N_CORES)

    xqT = nc.dram_tensor("xqT", [D, T], F16, kind="ExternalInput").ap()
    xkT = nc.dram_tensor("xkT", [D, T], F16, kind="ExternalInput").ap()
    xvT = nc.dram_tensor("xvT", [D, T], F16, kind="ExternalInput").ap()
    # per-pair packed weights, layout [128, 8*128]: chunk d at cols d*128
    wq = [nc.dram_tensor(f"wq{p}", [128, D], F16, kind="ExternalInput").ap()
          for p in range(2)]
    wk = [nc.dram_tensor(f"wk{p}", [128, D], F16, kind="ExternalInput").ap()
          for p in range(2)]
    # all-4-head V weights for token-major projection: chunk d at cols d*256
    wv4 = nc.dram_tensor("wv4", [128, ND * 256], F16, kind="ExternalInput").ap()
    wo = [nc.dram_tensor(f"wo{p}", [128, D], F16, kind="ExternalInput").ap()
          for p in range(2)]
    pout = nc.dram_tensor("pout", [T, D], F16, kind="ExternalOutput").ap()

    with tile.TileContext(nc) as tc:
        with (
            tc.tile_pool(name="consts", bufs=1) as consts,
            tc.tile_pool(name="persist", bufs=1) as persist,
            tc.tile_pool(name="xs", bufs=1) as xs,
            tc.tile_pool(name="at", bufs=24) as atp,
            tc.tile_pool(name="o2t", bufs=2) as o2tp,
            tc.tile_pool(name="os", bufs=2) as osp,
            tc.tile_pool(name="small", bufs=1) as smallp,
            tc.tile_pool(name="psS", bufs=2, space="PSUM") as psS,
            tc.tile_pool(name="psO", bufs=1, space="PSUM") as psO,
            tc.tile_pool(name="psX", bufs=2, space="PSUM") as psX,
        ):
            # ---- SBUF tiles ----
            wq_sb = [consts.tile([128, D], F16, tag=f"wq{p}", name=f"wq_sb{p}") for p in range(2)]
            wk_sb = [consts.tile([128, D], F16, tag=f"wk{p}", name=f"wk_sb{p}") for p in range(2)]
            wo_sb = [consts.tile([128, D], F16, tag=f"wo{p}", name=f"wo_sb{p}") for p in range(2)]
            wv4_sb = consts.tile([128, ND * 256], F16, tag="wv4", name="wv4_sb")
            qt = [[persist.tile([128, QC], F16, tag=f"qt{p}_{t}", name=f"qt{p}_{t}")
                   for t in range(NQ)] for p in range(2)]
            kt = [persist.tile([128, T], F16, tag=f"kt{p}", name=f"kt{p}") for p in range(2)]
            # token(key)-major [V | 1] per (head, key-block): [128, 65] each
            v2 = [[persist.tile([128, 65], BF16, tag=f"v2_{h}_{b}", name=f"v2_{h}_{b}")
                   for b in range(NKB)] for h in range(4)]
            ones_f32 = consts.tile([128, 1], F32, tag="ones", name="ones_f32")
            wu = consts.tile([128, 128], F16, tag="wu", name="wu")
            dummy_act = consts.tile([1, 1], BF16, tag="dummy", name="dummy_act")
            xk = [xs.tile([128, T], F16, tag=f"xk{d}", name=f"xk{d}") for d in range(ND)]
            xq = [xs.tile([128, T], F16, tag=f"xq{d}", name=f"xq{d}") for d in range(ND)]
            xv = [xs.tile([128, T], F16, tag=f"xv{d}", name=f"xv{d}") for d in range(ND)]

            # ---- t=0: warm-up + ACT table preload (ride the DMA wait) ----
            nc.vector.memset(wu[:], 0.125)
            nc.vector.memset(ones_f32[:], 1.0)
            nc.scalar.activation(dummy_act[:], wu[0:1, 0:1],
                                 mybir.ActivationFunctionType.Exp)
            wups = psX.tile([128, QC], F32, tag="x", name="warmup_ps")
            for i in range(44):
                nc.tensor.matmul(wups[:, 0:128], wu[:], wu[:],
                                 start=True, stop=True)

            # ---- DMA phase A: minimal set for the first S, striped over
            # 4 issue queues (each dma_start costs ~620ns of issue time) ----
            qengs = [nc.sync, nc.gpsimd, nc.scalar]

            def dmac(eng, dst, src):
                return eng.dma_start(dst, src)

            # K inputs strictly first (kproj leads), then Q
            phaseA = [(wk_sb[0][:], wk[0][:]), (wk_sb[1][:], wk[1][:])]
            for d in range(ND):
                phaseA.append((xk[d][:, 0:QC], xkT[d * 128:(d + 1) * 128, 0:QC]))
            phaseA.append((wq_sb[0][:], wq[0][:]))
            phaseA.append((wq_sb[1][:], wq[1][:]))
            for d in range(ND):
                phaseA.append((xq[d][:, 0:QC], xqT[d * 128:(d + 1) * 128, 0:QC]))
            lastA = {}
            for i, (dst, src) in enumerate(phaseA):
                lastA[i % 3] = dmac(qengs[i % 3], dst, src)

            # v2 ones columns (vector; overlaps remaining DMA wait)
            for h in range(4):
                for b in range(NKB):
                    nc.vector.tensor_copy(v2[h][b][:, 64:65], ones_f32[:])

            # ---- DMA phase B: deadline order, on sync+gpsimd only ----
            phaseB = []

            def addB(x, xT_, c0, c1):
                for d in range(ND):
                    phaseB.append((x[d][:, c0:c1], xT_[d * 128:(d + 1) * 128, c0:c1]))

            phaseB.append((wv4_sb[:], wv4[:]))
            addB(xk, xkT, QC, 2 * QC)          # kproj t1 chunks
            addB(xv, xvT, 0, QC)               # vproj kc0-3
            addB(xk, xkT, 2 * QC, 3 * QC)      # kproj t2
            addB(xk, xkT, 3 * QC, 4 * QC)      # kproj t3
            addB(xv, xvT, QC, 2 * QC)          # vproj kc4-7
            addB(xq, xqT, QC, 2 * QC)          # qproj(1,*)
            addB(xv, xvT, 2 * QC, 4 * QC)      # vproj kc8-15
            phaseB.append((wo_sb[0][:], wo[0][:]))
            phaseB.append((wo_sb[1][:], wo[1][:]))
            addB(xq, xqT, 2 * QC, 3 * QC)      # qproj(2,*)
            addB(xq, xqT, 3 * QC, 4 * QC)      # qproj(3,*)
            gated = set()
            for i, (dst, src) in enumerate(phaseB):
                eng = nc.sync if i % 2 == 0 else nc.gpsimd
                ins = dmac(eng, dst, src)
                if i < 2:
                    # phase B competes with phase A for HBM bandwidth;
                    # hold it back until the critical set has landed
                    for a in lastA.values():
                        add_dep_helper(ins.ins, a.ins, sync=True,
                                       reason="phaseB after phaseA")

            # ---- projection helpers ----
            def chain_pair(specs, pfx):
                """Full 2-interleaved accumulation chains (pre-phase only).
                specs: list of (x, w_sb_p, dst_ap, col_slice)."""
                pss = [psX.tile([128, QC], F32, tag="x", name=f"ps_{pfx}_{i}")
                       for i in range(len(specs))]
                for d in range(ND):
                    for i, (x, w_sb_p, dst, sl) in enumerate(specs):
                        nc.tensor.matmul(
                            pss[i][:], w_sb_p[:, d * 128:(d + 1) * 128],
                            x[d][:, sl], start=(d == 0), stop=(d == ND - 1))
                for i, (x, w_sb_p, dst, sl) in enumerate(specs):
                    nc.vector.tensor_copy(dst, pss[i][:])

            def proj_chain(x, w_sb_p, dst, sl, pfx):
                """Self-contained 8-matmul accumulation chain (one closure:
                alloc + matmuls + copy, so psX stays strictly sequential)."""
                def fn():
                    ps = psX.tile([128, QC], F32, tag="x", name=f"ps_{pfx}")
                    for d in range(ND):
                        nc.tensor.matmul(
                            ps[:], w_sb_p[:, d * 128:(d + 1) * 128],
                            x[d][:, sl], start=(d == 0), stop=(d == ND - 1))
                    nc.vector.tensor_copy(dst, ps[:])
                return fn

            def kproj1(p, t):
                sl = slice(t * QC, (t + 1) * QC)
                return proj_chain(xk, wk_sb[p], kt[p][:, sl], sl, f"k{p}{t}")

            def qproj1(qc_, p):
                sl = slice(qc_ * QC, (qc_ + 1) * QC)
                return proj_chain(xq, wq_sb[p], qt[p][qc_][:], sl, f"q{qc_}{p}")

            def emit_vproj(kc):
                """token-major V projection for key block kc: all 4 heads."""
                psv = psX.tile([128, 256], F32, tag="x", name=f"psv_{kc}")
                for d in range(ND):
                    nc.tensor.matmul(
                        psv[:], xv[d][:, kc * 128:(kc + 1) * 128],
                        wv4_sb[:, d * 256:(d + 1) * 256],
                        start=(d == 0), stop=(d == ND - 1))
                for h in range(4):
                    nc.vector.tensor_copy(v2[h][kc][:, 0:64],
                                          psv[:, h * 64:(h + 1) * 64])

            # ---- attention pipeline state ----
            at_all = {}     # (s, kb) -> at tile
            po_all = {}     # s -> [po_h0, po_h1]
            o2t_all = {}    # (qc, p) -> o2t tile
            os_live = {}

            def emit_av(s, kb):
                p = s % 2
                if kb == 0:
                    po_all[s] = [psO.tile([65, QC], F32, tag=f"o{h}",
                                          name=f"po_{s}_{h}") for h in range(2)]
                po = po_all[s]
                at_t = at_all[(s, kb)]
                for h in range(2):
                    nc.tensor.matmul(
                        po[h][:], v2[2 * p + h][kb][:],
                        at_t[:, h * QC:(h + 1) * QC],
                        start=(kb == 0), stop=(kb == NKB - 1))

            def emit_norm(s):
                qc_, p = s // 2, s % 2
                po = po_all[s]
                o2t_t = o2tp.tile([128, QC], F16, tag=f"o2t{p}",
                                  name=f"o2t_{qc_}_{p}")
                # interleave the two heads' chains so the gpsimd broadcasts
                # overlap the DVE work
                d_sb = [smallp.tile([1, QC], F32, tag=f"d{h}", name=f"d_{s}_{h}")
                        for h in range(2)]
                r = [smallp.tile([1, QC], F32, tag=f"r{h}", name=f"r_{s}_{h}")
                     for h in range(2)]
                rb = [smallp.tile([64, QC], F32, tag=f"rb{h}", name=f"rb_{s}_{h}")
                      for h in range(2)]
                for h in range(2):
                    nc.vector.tensor_copy(d_sb[h][:], po[h][64:65, :])
                for h in range(2):
                    nc.vector.reciprocal_approx_fast(r[h][:], d_sb[h][:])
                for h in range(2):
                    nc.gpsimd.partition_broadcast(rb[h][:], r[h][:])
                for h in range(2):
                    nc.vector.tensor_mul(
                        o2t_t[h * 64:(h + 1) * 64, :],
                        po[h][0:64, :], rb[h][:])
                o2t_all[(qc_, p)] = o2t_t
                del po_all[s]

            def emit_outproj_group(oqc, sub, oc):
                q0 = oqc * QC
                if oc == 0:
                    os_live[(oqc, sub)] = osp.tile(
                        [128, D], F16, tag="os", name=f"os_{oqc}_{sub}")
                ost = os_live[(oqc, sub)]
                pp = psX.tile([128, 512], F32, tag="x", name=f"pp_{oqc}_{sub}_{oc}")
                for p in range(2):
                    nc.tensor.matmul(
                        pp[:],
                        o2t_all[(oqc, p)][:, sub * 128:(sub + 1) * 128],
                        wo_sb[p][:, oc * 512:(oc + 1) * 512],
                        start=(p == 0), stop=(p == 1))
                nc.vector.tensor_copy(ost[:, oc * 512:(oc + 1) * 512], pp[:])
                if oc == 1:
                    nc.sync.dma_start(
                        pout[q0 + sub * 128:q0 + (sub + 1) * 128, :],
                        ost[:])
                    del os_live[(oqc, sub)]

            # ---- pre-phase: K chunk0 + Q chunk0 for both pairs ----
            chain_pair([(xk, wk_sb[0], kt[0][:, 0:QC], slice(0, QC)),
                        (xk, wk_sb[1], kt[1][:, 0:QC], slice(0, QC))], "kpre")
            chain_pair([(xq, wq_sb[0], qt[0][0][:], slice(0, QC)),
                        (xq, wq_sb[1], qt[1][0][:], slice(0, QC))], "qpre")

            # ---- insert schedule ----
            from collections import defaultdict
            inserts = defaultdict(list)

            def put(s, kb, fn):
                inserts[(s, kb)].append(fn)

            # step 0: remaining K chunks (atomic chains, DMA-paced)
            # + first V blocks
            for i, (p, t) in enumerate([(0, 1), (0, 2), (0, 3),
                                        (1, 1), (1, 2), (1, 3)]):
                put(0, 2 * i + 2, kproj1(p, t))
            put(0, 13, lambda: emit_vproj(0))
            put(0, 14, lambda: emit_vproj(1))
            put(0, 15, lambda: emit_vproj(2))
            # step 1: V blocks (xv DMA paced), AV(0) front, qproj for step 2
            put(1, 0, lambda: emit_vproj(3))
            for i, kc in enumerate(range(4, 10)):
                put(1, 4 + 2 * i, lambda kc=kc: emit_vproj(kc))
            for j in range(8):
                put(1, j + 3, lambda j=j: emit_av(0, j))
            put(1, 13, qproj1(1, 0))
            # step 2: last V blocks, AV(0) tail + norm(0), AV(1), qproj
            for i, kc in enumerate(range(10, 16)):
                put(2, i, lambda kc=kc: emit_vproj(kc))

            def av0_tail(lo, hi, with_norm):
                def fn():
                    for j in range(lo, hi):
                        emit_av(0, j)
                    if with_norm:
                        emit_norm(0)
                return fn
            put(1, 15, av0_tail(8, 10, False))
            put(2, 1, av0_tail(10, 12, False))
            put(2, 3, av0_tail(12, 14, False))
            put(2, 5, av0_tail(14, 16, True))
            for j in range(14):
                put(2, 7 + j // 2, lambda j=j: emit_av(1, j))

            def av1_tail():
                for j in range(14, NKB):
                    emit_av(1, j)
                emit_norm(1)
            put(2, 15, av1_tail)
            put(2, 14, qproj1(1, 1))
            # step 3: AV(2) mid-step (po frees after norm(1) lands),
            # outproj(0) back half
            for j in range(16):
                put(3, 3 + j // 2, lambda j=j: emit_av(2, j))
            put(3, 11, lambda: emit_norm(2))
            put(3, 12, qproj1(2, 0))
            # steps 4..7: AV(s-1) dense up front (2/slot), norm at slot 8
            for s in range(4, 8):
                for j in range(16):
                    put(s, j // 2, lambda s=s, j=j: emit_av(s - 1, j))
                put(s, 8, lambda s=s: emit_norm(s - 1))
            for s, (qc_, p) in {4: (2, 1), 5: (3, 0), 6: (3, 1)}.items():
                put(s, 11, qproj1(qc_, p))
            # outproj(0): 5 in step 3, 3 in step 4; outproj(1): 6 in
            # step 5, 2 in step 6; outproj(2) fully in step 7;
            # outproj(3) in the tail
            groups = [(i, j) for i in range(4) for j in range(2)]
            for oqc in range(3):
                for g, (sub, oc) in enumerate(groups):
                    if oqc == 0:
                        if g < 5:
                            s, slot = 3, (11, 12, 13, 14, 15)[g]
                        else:
                            s, slot = 4, (9, 10, 12)[g - 5]
                    elif oqc == 1:
                        if g < 6:
                            s, slot = 5, (9, 10, 12, 13, 14, 15)[g]
                        else:
                            s, slot = 6, (9, 12)[g - 6]
                    else:
                        s, slot = 7, (9, 10, 11, 12, 13, 14, 15, 15)[g]
                    put(s, slot, lambda oqc=oqc, sub=sub, oc=oc:
                        emit_outproj_group(oqc, sub, oc))

            # ---- 8-step S/exp pipeline with slotted inserts ----
            steps = [(qc_, p) for qc_ in range(NQ) for p in range(2)]
            for s, (qc_, p) in enumerate(steps):
                for kb in range(NKB):
                    k0 = kb * KB
                    ps = psS.tile([128, 2 * QC], F32, tag="s", name=f"s_{s}_{kb}")
                    nc.tensor.matmul(
                        ps[:, 0:QC],
                        kt[p][0:64, k0:k0 + KB],
                        qt[p][qc_][0:64, :],
                        start=True, stop=True, tile_position=(0, 0))
                    nc.tensor.matmul(
                        ps[:, QC:2 * QC],
                        kt[p][64:128, k0:k0 + KB],
                        qt[p][qc_][64:128, :],
                        start=True, stop=True, tile_position=(64, 0))
                    at_t = atp.tile([128, 2 * QC], BF16, tag="at",
                                    name=f"at_{s}_{kb}")
                    nc.scalar.activation(
                        at_t[:], ps[:], mybir.ActivationFunctionType.Exp)
                    at_all[(s, kb)] = at_t
                    for fn in inserts.get((s, kb), ()):
                        fn()

            # ---- tail: AV + norm of step 7, then outproj(3) ----
            for j in range(NKB):
                emit_av(7, j)
            emit_norm(7)
            for sub in range(4):
                for oc in range(2):
                    emit_outproj_group(3, sub, oc)

    nc.compile()
    nc.m = get_hw_module(nc.m)
    return nc


def _pack_w(w_pair):
    # w_pair: [2, 1024, 64] -> [1024, 128] -> chunk-major [128, 8*128]
    w = np.concatenate([w_pair[0], w_pair[1]], axis=1)          # [1024, 128]
    return np.ascontiguousarray(
        w.reshape(ND, 128, 128).transpose(1, 0, 2).reshape(128, D))


def _pack_wv4(w_quad):
    # w_quad: [4, 1024, 64] -> [1024, 256] -> chunk-major [128, 8*256]
    w = np.concatenate([w_quad[h] for h in range(4)], axis=1)   # [1024, 256]
    return np.ascontiguousarray(
        w.reshape(ND, 128, 256).transpose(1, 0, 2).reshape(128, ND * 256))


def _pack_wo(wo_pair):
    # wo_pair: [2, 64, 1024] -> [128, 1024]
    return np.ascontiguousarray(np.concatenate([wo_pair[0], wo_pair[1]], axis=0))


def kernel(q, k, v, W_query, W_key, W_val, W_out, _trace=False):
    q = np.asarray(q, dtype=np.float32)
    k = np.asarray(k, dtype=np.float32)
    v = np.asarray(v, dtype=np.float32)
    W_query = np.asarray(W_query, dtype=np.float32)
    W_key = np.asarray(W_key, dtype=np.float32)
    W_val = np.asarray(W_val, dtype=np.float32)
    W_out = np.asarray(W_out, dtype=np.float32)

    if "nc" not in _CACHE:
        _CACHE["nc"] = _build()
    nc = _CACHE["nc"]

    norm = 1.0 / np.sqrt(E)
    xT = {}
    for b in range(2):
        xT[("q", b)] = np.ascontiguousarray(q[b].T).astype(np.float16)
        xT[("k", b)] = np.ascontiguousarray(k[b].T).astype(np.float16)
        xT[("v", b)] = np.ascontiguousarray(v[b].T).astype(np.float16)

    in_maps = []
    for c in range(N_CORES):
        b, g = c // 4, c % 4
        hs = [4 * g, 4 * g + 1, 4 * g + 2, 4 * g + 3]
        m = {
            "xqT": xT[("q", b)], "xkT": xT[("k", b)], "xvT": xT[("v", b)],
            "wv4": _pack_wv4(W_val[hs]).astype(np.float16),
        }
        for p in range(2):
            hp = hs[2 * p:2 * p + 2]
            m[f"wq{p}"] = _pack_w(W_query[hp] * norm).astype(np.float16)
            m[f"wk{p}"] = _pack_w(W_key[hp]).astype(np.float16)
            m[f"wo{p}"] = _pack_wo(W_out[hp]).astype(np.float16)
        in_maps.append(m)

    res = run_bass_kernel_spmd(nc, in_maps, list(range(N_CORES)),
                               trace=_trace)
    parts = [res.results[c]["pout"].astype(np.float32) for c in range(N_CORES)]
    out = np.stack([
        parts[0] + parts[1] + parts[2] + parts[3],
        parts[4] + parts[5] + parts[6] + parts[7],
    ])
    if _trace:
        _CACHE["last_result"] = res
    return out
